# revision 2
# baseline (speedup 1.0000x reference)
"""Trainium2 kernel for nn_ClusterMemory (cross-entropy over a 100k-row memory bank).

Computes: mean_b[ logsumexp_c(x_b . f_c / T) - x_b . f_{t_b} / T ]
for x [1024, 256], f [100000, 256] (unit-norm rows), T = 0.05.

Sharding: the memory bank (and therefore the logits) is split along the
class dimension across 8 NeuronCores (12500 classes each, zero-padded to
12544 = 98*128). Each core computes partial sum_c exp(logit - C_b) for its
classes with a per-sample fixed shift C_b = 6*||x_b|| (a tight upper-bound
estimate of the per-sample max logit for unit-norm bank rows; exp has ~85
orders of magnitude of fp32 headroom either way, so no max pass is needed;
a host-side retry adjusts the shift in the astronomically unlikely event of
overflow/underflow). The target-row dot products land on the core that owns
each target row (host pre-gathers the owned rows; non-owned rows are zero).
The host only combines the [8, 1024] partial sums: lse = C + log(sum_d s_d),
nll = lse - t, output = mean(nll).

On-core dataflow (per 1792-class supertile, 7 per core):
  DMA:  featT tile [128f, 1792c] bf16 x2 (k-chunks), contiguous loads
  PE:   psum[128b, 1792c] += xT_chunk[128f,128b].T @ featT[128f, :] (bf16)
  ACT:  exp(20*psum - C_b) in-place, accum_out = per-sample row-sum
"""

import numpy as np
import ml_dtypes

from concourse import bacc, tile
from concourse import mybir
from concourse.bass_utils import run_bass_kernel_spmd

# Problem geometry (hardcoded per contract).
B = 1024          # batch
F = 256           # features
C_TOTAL = 100000  # memory bank rows
N_CORES = 8
C_SHARD = C_TOTAL // N_CORES     # 12500
C_PAD = 12544                    # 98 * 128
# class supertiles: one 256 tail first (fast pipeline fill), then six of
# 2048 (4 PSUM banks each)
CS_SIZES = [256] + [2048] * 6
CS_OFFS = [sum(CS_SIZES[:i]) for i in range(len(CS_SIZES))]
N_CS = len(CS_SIZES)             # 7
N_BT = B // 128                  # 8 batch tiles
INV_TEMP = 20.0                  # 1 / 0.05

LAST_EXEC_NS = None

_CACHED_NC = None


def _build_nc(repeat=1):
    nc = bacc.Bacc("TRN2", target_bir_lowering=False, debug=False,
                   num_devices=N_CORES)
    bf16 = mybir.dt.bfloat16
    f32 = mybir.dt.float32

    featT = nc.dram_tensor("featT", [F, C_PAD], bf16, kind="ExternalInput")
    xT = nc.dram_tensor("xT", [F, B], bf16, kind="ExternalInput")
    x32 = nc.dram_tensor("x32", [128, N_BT * F], f32, kind="ExternalInput")
    tgt32 = nc.dram_tensor("tgt32", [128, N_BT * F], f32, kind="ExternalInput")
    biasneg = nc.dram_tensor("biasneg", [128, N_BT], f32, kind="ExternalInput")
    s_stats = nc.dram_tensor("s_stats", [128, N_CS * N_BT], f32,
                             kind="ExternalOutput")
    t_dots = nc.dram_tensor("t_dots", [128, N_BT], f32, kind="ExternalOutput")


    import contextlib
    with tile.TileContext(nc) as tc:
        with tc.tile_pool(name="const", bufs=1) as const, \
             tc.tile_pool(name="feat", bufs=3) as feat, \
             tc.tile_pool(name="ps", bufs=2, space="PSUM") as psp, \
             tc.tile_pool(name="misc", bufs=1) as misc, \
             (tc.For_i(0, repeat, 1) if repeat > 1
              else contextlib.nullcontext()):

            # One-time loads (bias first: the warmup exp only needs it).
            bias_t = const.tile([128, N_BT], f32)
            nc.sync.dma_start(out=bias_t[:], in_=biasneg.ap()[:])
            xT0 = const.tile([128, B], bf16)
            nc.sync.dma_start(out=xT0[:], in_=xT.ap()[0:128, :])
            xT1 = const.tile([128, B], bf16)
            nc.sync.dma_start(out=xT1[:], in_=xT.ap()[128:256, :])

            # Warmup exp so the ACT table load overlaps the first featT DMA
            # instead of serializing before the first real exp op.
            warm = misc.tile([128, 1], f32)
            nc.scalar.activation(warm[:], bias_t[:, 0:1],
                                 mybir.ActivationFunctionType.Exp)

            s_acc = const.tile([128, N_CS * N_BT], f32)
            t_acc = const.tile([128, N_BT], f32)

            # Main loop: stream the bank, accumulate exp row-sums.
            for cs in range(N_CS):
                cs_w = CS_SIZES[cs]
                csl = slice(CS_OFFS[cs], CS_OFFS[cs] + cs_w)
                fT0 = feat.tile([128, cs_w], bf16, tag="fT0")
                nc.sync.dma_start(out=fT0[:], in_=featT.ap()[0:128, csl])
                fT1 = feat.tile([128, cs_w], bf16, tag="fT1")
                nc.sync.dma_start(out=fT1[:], in_=featT.ap()[128:256, csl])
                cc_chunks = [(c, min(512, cs_w - c))
                             for c in range(0, cs_w, 512)]
                for bt in range(N_BT):
                    ps = psp.tile([128, cs_w], f32, tag="ps")
                    bsl = slice(bt * 128, (bt + 1) * 128)
                    for (c0, cw) in cc_chunks:
                        nc.tensor.matmul(
                            ps[:, c0:c0 + cw], lhsT=xT0[:, bsl],
                            rhs=fT0[:, c0:c0 + cw], start=True, stop=False)
                        nc.tensor.matmul(
                            ps[:, c0:c0 + cw], lhsT=xT1[:, bsl],
                            rhs=fT1[:, c0:c0 + cw], start=False, stop=True)
                    # exp output lands in SBUF scratch (discarded): avoids a
                    # same-bank PSUM read+write every cycle on ScalarE.
                    eo = misc.tile([128, cs_w], bf16, tag="eo")
                    nc.scalar.activation(
                        eo[:], ps[:], mybir.ActivationFunctionType.Exp,
                        bias=bias_t[:, bt:bt + 1], scale=INV_TEMP,
                        accum_out=s_acc[:, cs * N_BT + bt:cs * N_BT + bt + 1],
                    )

            # Target-row dot products (DVE, fp32): t = sum_f x * f_tgt.
            # Emitted after the main loop so their DMAs don't delay the
            # first featT supertile; DVE is otherwise idle and the ops are
            # dependency-free, so the scheduler runs them during the loop.
            x_nat = const.tile([128, N_BT * F], f32)
            nc.sync.dma_start(out=x_nat[:], in_=x32.ap()[:])
            tgt_nat = const.tile([128, N_BT * F], f32)
            nc.sync.dma_start(out=tgt_nat[:], in_=tgt32.ap()[:])
            for bt in range(N_BT):
                sl = slice(bt * F, (bt + 1) * F)
                prod = misc.tile([128, F], f32)
                nc.vector.tensor_mul(prod[:], x_nat[:, sl], tgt_nat[:, sl])
                nc.vector.reduce_sum(
                    t_acc[:, bt:bt + 1], prod[:], axis=mybir.AxisListType.X)

            nc.sync.dma_start(out=s_stats.ap()[:], in_=s_acc[:])
            nc.sync.dma_start(out=t_dots.ap()[:], in_=t_acc[:])
    nc.compile()
    return nc


def _get_nc():
    global _CACHED_NC
    if _CACHED_NC is None:
        _CACHED_NC = _build_nc()
    return _CACHED_NC


def _run(in_maps, trace=False):
    global LAST_EXEC_NS
    nc = _get_nc()
    res = run_bass_kernel_spmd(nc, in_maps, core_ids=list(range(N_CORES)),
                               trace=trace)
    if res.exec_time_ns is not None:
        LAST_EXEC_NS = res.exec_time_ns
    return res.results


def _pview(a):
    # [128, N_BT]-shaped view (partition p, batch-tile bt) <-> b = bt*128 + p.
    return np.ascontiguousarray(a.reshape(N_BT, 128).T)


def prep_in_maps(x, tgt, feats):
    """Build the per-core input maps. Returns (in_maps, c_shift)."""
    # Per-sample exp shift: tight estimate of max_c logit for unit-norm rows.
    xnorm = np.linalg.norm(x.astype(np.float64), axis=1)
    c_shift = (6.0 * xnorm).astype(np.float32)           # [B]

    xT_np = np.ascontiguousarray(x.T).astype(ml_dtypes.bfloat16)
    x32_np = np.ascontiguousarray(
        x.reshape(N_BT, 128, F).transpose(1, 0, 2).reshape(128, N_BT * F))

    owner = tgt // C_SHARD                                # [B] in [0, 8)
    tgt_rows_all = feats[tgt]                             # [B, F] fp32

    in_maps = []
    for d in range(N_CORES):
        shard = feats[d * C_SHARD:(d + 1) * C_SHARD]
        featT_np = np.zeros((F, C_PAD), dtype=ml_dtypes.bfloat16)
        featT_np[:, :C_SHARD] = shard.T.astype(ml_dtypes.bfloat16)
        tgt_rows = np.where((owner == d)[:, None], tgt_rows_all, 0.0)
        tgt32_np = np.ascontiguousarray(
            tgt_rows.reshape(N_BT, 128, F).transpose(1, 0, 2)
            .reshape(128, N_BT * F).astype(np.float32))
        in_maps.append({
            "featT": featT_np,
            "xT": xT_np,
            "x32": x32_np,
            "tgt32": tgt32_np,
            "biasneg": -_pview(c_shift),
        })
    return in_maps, c_shift


def kernel(inputs, targets, features, _trace=False):
    x = np.ascontiguousarray(np.asarray(inputs, dtype=np.float32))
    tgt = np.asarray(targets).astype(np.int64)
    feats = np.asarray(features, dtype=np.float32)
    assert x.shape == (B, F) and tgt.shape == (B,) and feats.shape == (C_TOTAL, F)

    in_maps, c_shift = prep_in_maps(x, tgt, feats)

    shift_pv = _pview(c_shift).astype(np.float64)         # [128, N_BT]
    for attempt in range(3):
        results = _run(in_maps, trace=_trace)
        s_pv = np.zeros((128, N_BT), dtype=np.float64)
        t_pv = np.zeros((128, N_BT), dtype=np.float64)
        for d in range(N_CORES):
            st = results[d]["s_stats"].astype(np.float64)
            s_pv += st.reshape(128, N_CS, N_BT).sum(axis=1)
            t_pv += results[d]["t_dots"].astype(np.float64)
        good = np.isfinite(s_pv) & (s_pv > 0.0)
        if good.all():
            break
        # Shift was off for some sample (never expected for this data
        # distribution) - adjust and retry.
        delta = np.where(np.isinf(s_pv), 60.0, np.where(s_pv <= 0, -60.0, 0.0))
        shift_pv = shift_pv + delta
        for d in range(N_CORES):
            in_maps[d]["biasneg"] = (-shift_pv).astype(np.float32)

    lse = shift_pv + np.log(s_pv)
    nll = lse - INV_TEMP * t_pv
    return np.float32(nll.mean())


if __name__ == "__main__":
    rng = np.random.default_rng(0)
    x = rng.standard_normal((B, F)).astype(np.float32)
    t = rng.integers(0, C_TOTAL, B)
    f = rng.standard_normal((C_TOTAL, F)).astype(np.float32)
    f /= np.linalg.norm(f, axis=1, keepdims=True)
    out = kernel(x, t, f)
    print("kernel out:", out)



# revision 3
# speedup vs baseline: 3.4975x; 3.4975x over previous
"""Trainium2 kernel for nn_ClusterMemory (cross-entropy over a 100k-row memory bank).

Computes: mean_b[ logsumexp_c(x_b . f_c / T) - x_b . f_{t_b} / T ]
for x [1024, 256], f [100000, 256] (unit-norm rows), T = 0.05.

Sharding: the memory bank (and therefore the logits) is split along the
class dimension across 8 NeuronCores (12500 classes each, zero-padded to
12544 = 24.5*512). Each core computes partial sum_c exp(logit - C_b) for
its classes with a per-sample fixed shift C_b = 6*||x_b|| (fp32 exp absorbs
the data's aligned pairs, which exceed the shift by up to +67; a host-side
retry adjusts the shift in the event of overflow/underflow). The target-row
dot products land on the core that owns each target row (host pre-gathers
the owned rows; non-owned rows are zero). The host combines the [8, ...]
partial sums: lse = C + log(sum_d s_d), nll = lse - t, output = mean(nll).

v2: the matmul runs in fp8 e4m3 with DoubleRow perf mode (contraction 256
in one pass, HW-measured ~215-285 ns per 512-col MM vs 2x that for bf16).
The inputs are pre-scaled on host (x*20, f*369.33) so psum = 369.33*logit;
scaling is ~lossless for fp8's relative grid and keeps everything within
e4m3's +-240 range (|20x| <= ~96, |369.33f| <= ~121). ScalarE consumes each
psum supertile with a single fused exp+row-sum (HW-measured 553 ns per
128x2048 tile). End-to-end fp8 quantization error on this data: ~1e-3 rel.

On-core dataflow (per 2048-class supertile, 6 + one 256 tail per core):
  DMA:  featT tile [128p, 2k, 2048c] fp8, 2x2KB contiguous per partition
  PE:   psum[128b, 512c] = DoubleRow MM (xT8[:, :, bt], featT[:, :, c-chunk])
  ACT:  exp(psum/369.33 - C_b), accum_out -> s_stats column
  DVE:  target-row dot products (idle engine otherwise)
"""

import numpy as np
import ml_dtypes

from concourse import bacc, tile
from concourse import mybir
from concourse.bass_utils import run_bass_kernel_spmd

# Problem geometry (hardcoded per contract).
B = 1024          # batch
F = 256           # features
C_TOTAL = 100000  # memory bank rows
N_CORES = 8
C_SHARD = C_TOTAL // N_CORES     # 12500
C_PAD = 12544                    # 24.5 * 512
# class supertiles: one 256 tail first (fast pipeline fill), then six of 2048
CS_SIZES = [256] + [2048] * 6
CS_OFFS = [sum(CS_SIZES[:i]) for i in range(len(CS_SIZES))]
N_CS = len(CS_SIZES)             # 7
N_BT = B // 128                  # 8 batch tiles
INV_TEMP = 20.0                  # 1 / 0.05
A_SCALE = 256.0 / float(np.log(2.0))   # 369.33; psum = A_SCALE * logit
X_S = 20.0                       # host scale on x (folds INV_TEMP)
F_S = A_SCALE                    # host scale on f

LAST_EXEC_NS = None

_CACHED_NC = None


def _build_nc(repeat=1):
    nc = bacc.Bacc("TRN2", target_bir_lowering=False, debug=False,
                   num_devices=N_CORES)
    f8 = mybir.dt.float8e4
    f32 = mybir.dt.float32
    bf16 = mybir.dt.bfloat16

    fT8 = nc.dram_tensor("fT8", [128, 2, C_PAD], f8, kind="ExternalInput")
    xT8 = nc.dram_tensor("xT8", [128, 2, B], f8, kind="ExternalInput")
    x32 = nc.dram_tensor("x32", [128, N_BT * F], f32, kind="ExternalInput")
    tgt32 = nc.dram_tensor("tgt32", [128, N_BT * F], f32, kind="ExternalInput")
    biasneg = nc.dram_tensor("biasneg", [128, N_BT], f32, kind="ExternalInput")
    s_stats = nc.dram_tensor("s_stats", [128, N_CS * N_BT], f32,
                             kind="ExternalOutput")
    t_dots = nc.dram_tensor("t_dots", [128, N_BT], f32, kind="ExternalOutput")

    import contextlib
    with tile.TileContext(nc) as tc:
        with tc.tile_pool(name="const", bufs=1) as const, \
             tc.tile_pool(name="feat", bufs=3) as feat, \
             tc.tile_pool(name="ps", bufs=2, space="PSUM") as psp, \
             tc.tile_pool(name="misc", bufs=1) as misc, \
             (tc.For_i(0, repeat, 1) if repeat > 1
              else contextlib.nullcontext()):

            # One-time loads (bias first: the warmup exp only needs it).
            bias_t = const.tile([128, N_BT], f32)
            nc.sync.dma_start(out=bias_t[:], in_=biasneg.ap()[:])
            xt = const.tile([128, 2, B], f8)
            nc.sync.dma_start(out=xt[:], in_=xT8.ap()[:])

            # Warmup exp so the ACT table load overlaps the first featT DMA
            # instead of serializing before the first real exp op.
            warm = misc.tile([128, 1], f32)
            nc.scalar.activation(warm[:], bias_t[:, 0:1],
                                 mybir.ActivationFunctionType.Exp)

            s_acc = const.tile([128, N_CS * N_BT], f32)

            # Main loop: stream the bank, accumulate exp row-sums.
            for cs in range(N_CS):
                cs_w = CS_SIZES[cs]
                csl = slice(CS_OFFS[cs], CS_OFFS[cs] + cs_w)
                fT = feat.tile([128, 2, cs_w], f8, tag="fT")
                nc.sync.dma_start(out=fT[:], in_=fT8.ap()[:, :, csl])
                cc_chunks = [(c, min(512, cs_w - c))
                             for c in range(0, cs_w, 512)]
                for bt in range(N_BT):
                    ps = psp.tile([128, cs_w], f32, tag="ps")
                    bsl = slice(bt * 128, (bt + 1) * 128)
                    for (c0, cw) in cc_chunks:
                        nc.tensor.matmul(
                            ps[:, c0:c0 + cw], lhsT=xt[:, :, bsl],
                            rhs=fT[:, :, c0:c0 + cw], start=True, stop=True,
                            perf_mode=mybir.MatmulPerfMode.DoubleRow)
                    # exp output lands in SBUF scratch (discarded); accum_out
                    # is the per-sample row-sum for this supertile.
                    eo = misc.tile([128, cs_w], bf16, tag="eo")
                    nc.scalar.activation(
                        eo[:], ps[:], mybir.ActivationFunctionType.Exp,
                        bias=bias_t[:, bt:bt + 1], scale=1.0 / A_SCALE,
                        accum_out=s_acc[:, cs * N_BT + bt:cs * N_BT + bt + 1],
                    )

            # Target-row dot products (DVE, fp32): t = sum_f x * f_tgt.
            # Emitted after the main loop so their DMAs don't delay the
            # first featT supertile; DVE is otherwise idle and the ops are
            # dependency-free, so the scheduler runs them during the loop.
            t_acc = const.tile([128, N_BT], f32)
            x_nat = const.tile([128, N_BT * F], f32)
            nc.sync.dma_start(out=x_nat[:], in_=x32.ap()[:])
            tgt_nat = const.tile([128, N_BT * F], f32)
            nc.sync.dma_start(out=tgt_nat[:], in_=tgt32.ap()[:])
            for bt in range(N_BT):
                sl = slice(bt * F, (bt + 1) * F)
                prod = misc.tile([128, F], f32)
                nc.vector.tensor_mul(prod[:], x_nat[:, sl], tgt_nat[:, sl])
                nc.vector.reduce_sum(
                    t_acc[:, bt:bt + 1], prod[:], axis=mybir.AxisListType.X)

            nc.sync.dma_start(out=s_stats.ap()[:], in_=s_acc[:])
            nc.sync.dma_start(out=t_dots.ap()[:], in_=t_acc[:])
    nc.compile()
    return nc


def _get_nc():
    global _CACHED_NC
    if _CACHED_NC is None:
        _CACHED_NC = _build_nc()
    return _CACHED_NC


def _run(in_maps, trace=False):
    global LAST_EXEC_NS
    nc = _get_nc()
    res = run_bass_kernel_spmd(nc, in_maps, core_ids=list(range(N_CORES)),
                               trace=trace)
    if res.exec_time_ns is not None:
        LAST_EXEC_NS = res.exec_time_ns
    return res.results


def _pview(a):
    # [128, N_BT]-shaped view (partition p, batch-tile bt) <-> b = bt*128 + p.
    return np.ascontiguousarray(a.reshape(N_BT, 128).T)


def _pack_T8(m, scale):
    """[R, 256] fp32 row-major -> [128, 2, R] fp8 with f = p + 128j."""
    t = (m.T * scale)                       # [256, R]
    t = np.clip(t, -240.0, 240.0)
    return np.ascontiguousarray(
        t.reshape(2, 128, -1).transpose(1, 0, 2)).astype(ml_dtypes.float8_e4m3)


def prep_in_maps(x, tgt, feats):
    """Build the per-core input maps. Returns (in_maps, c_shift)."""
    # Per-sample exp shift: estimate of max_c logit for unit-norm rows
    # (the data's planted aligned pairs exceed it by up to +67; fp32 exp
    # absorbs that, and the host retry below is the safety net).
    xnorm = np.linalg.norm(x.astype(np.float64), axis=1)
    c_shift = (6.0 * xnorm).astype(np.float32)           # [B]

    xT8_np = _pack_T8(x, X_S)                            # [128, 2, B]
    x32_np = np.ascontiguousarray(
        x.reshape(N_BT, 128, F).transpose(1, 0, 2).reshape(128, N_BT * F))

    owner = tgt // C_SHARD                                # [B] in [0, 8)
    tgt_rows_all = feats[tgt]                             # [B, F] fp32

    in_maps = []
    for d in range(N_CORES):
        shard = feats[d * C_SHARD:(d + 1) * C_SHARD]
        shard_pad = np.zeros((C_PAD, F), dtype=np.float32)
        shard_pad[:C_SHARD] = shard
        fT8_np = _pack_T8(shard_pad, F_S)                 # [128, 2, C_PAD]
        tgt_rows = np.where((owner == d)[:, None], tgt_rows_all, 0.0)
        tgt32_np = np.ascontiguousarray(
            tgt_rows.reshape(N_BT, 128, F).transpose(1, 0, 2)
            .reshape(128, N_BT * F).astype(np.float32))
        in_maps.append({
            "fT8": fT8_np,
            "xT8": xT8_np,
            "x32": x32_np,
            "tgt32": tgt32_np,
            "biasneg": -_pview(c_shift),
        })
    return in_maps, c_shift


def kernel(inputs, targets, features, _trace=False):
    x = np.ascontiguousarray(np.asarray(inputs, dtype=np.float32))
    tgt = np.asarray(targets).astype(np.int64)
    feats = np.asarray(features, dtype=np.float32)
    assert x.shape == (B, F) and tgt.shape == (B,) and feats.shape == (C_TOTAL, F)

    in_maps, c_shift = prep_in_maps(x, tgt, feats)

    shift_pv = _pview(c_shift).astype(np.float64)         # [128, N_BT]
    for attempt in range(3):
        results = _run(in_maps, trace=_trace)
        s_pv = np.zeros((128, N_BT), dtype=np.float64)
        t_pv = np.zeros((128, N_BT), dtype=np.float64)
        for d in range(N_CORES):
            st = results[d]["s_stats"].astype(np.float64)
            s_pv += st.reshape(128, N_CS, N_BT).sum(axis=1)
            t_pv += results[d]["t_dots"].astype(np.float64)
        good = np.isfinite(s_pv) & (s_pv > 0.0)
        if good.all():
            break
        # Shift was off for some sample - adjust and retry.
        delta = np.where(np.isinf(s_pv) | np.isnan(s_pv), 60.0,
                         np.where(s_pv <= 0, -60.0, 0.0))
        shift_pv = shift_pv + delta
        for d in range(N_CORES):
            in_maps[d]["biasneg"] = (-shift_pv).astype(np.float32)

    lse = shift_pv + np.log(s_pv)
    nll = lse - INV_TEMP * t_pv
    return np.float32(nll.mean())


if __name__ == "__main__":
    rng = np.random.default_rng(0)
    x = rng.standard_normal((B, F)).astype(np.float32)
    t = rng.integers(0, C_TOTAL, B)
    f = rng.standard_normal((C_TOTAL, F)).astype(np.float32)
    f /= np.linalg.norm(f, axis=1, keepdims=True)
    out = kernel(x, t, f)
    print("kernel out:", out)
